# revision 1
# baseline (speedup 1.0000x reference)
"""Causal self-attention (RMSNorm-QK + RoPE) Trainium2 kernel, 8-way
head-sharded SPMD.

Math (B=1, T=4096, D=2048, H=16, HD=128):
    q = rmsnorm(x @ Wq + bq) * gq ; k likewise ; v = x @ Wv + bv
    rq, rk = rope(q), rope(k)  (adjacent-pair rotation, freqs [T, HD/2])
    out = causal_softmax(rq rk^T / sqrt(HD)) @ v ; return out @ Wo + bo

Sharding: 2 heads per core (16 heads / 8 cores). The only cross-head
coupling is the RMSNorm mean-of-squares over all 2048 channels -> two
tiny [2, T/2] AllReduces (split in halves so the second half of the
projection phase hides the first collective's latency). Each core emits
a partial output (its heads' slice of the Wo contraction); the host
sums the 8 partials and adds bo.

On-device layout notes:
  - x is passed transposed (xT [D, T]); q/k are computed directly in
    transposed per-head layout yq/yk [128(hd), NH, T] so attention
    scores can be built as scoresT[k, q] = rk^T . rq with hd as the
    contraction dim (keys on partitions, queries on the free dim).
  - gq/gk are folded into Wq/Wk/bq/bk on the host; the pre-norm sum of
    squares is recovered with a (1/g^2)-weighted partition-sum matmul.
  - 1/sqrt(HD) and the rmsnorm scale fold into one per-t factor applied
    to q (and the k rmsnorm scale to k) after RoPE.
  - softmax skips the max-subtraction: scores for this problem are
    bounded (|s| < ~7, fp32 exp is safe), so exp / (ones-matmul column
    sum) is exact within fp32.
  - causal masking: only key tiles at/below the diagonal are computed;
    the 4 diagonal key-tiles of each q-tile are masked in-place with
    gpsimd.affine_select (zero-fill on the exp'd tile).
  - all matmul operands are float32r (full-rate on the PE at N>=256,
    ~2^-12 operand rounding), accumulation stays fp32 in PSUM.
  - the output projection for q-tile j is emitted after attention for
    q-tile j+1 so the PE never waits on the softmax epilogue (DVE).
"""

import math
import os
import numpy as np
from contextlib import ExitStack

import concourse.bass as bass
import concourse.tile as tile
from concourse import bacc, mybir
from concourse.bass_utils import run_bass_kernel_spmd

F32 = mybir.dt.float32
F32R = mybir.dt.float32r
AF = mybir.ActivationFunctionType

T_FULL = 4096
D = 2048
H = 16
HD = 128
NCORES = 8
NH = H // NCORES          # heads per core (2)
HW = NH * HD              # per-core head width (256)
P = 128
QT = 512                  # q tile (matmul free dim)
NKC = D // P              # 16 chunks of the D contraction
EPS = 1e-6

_NC_CACHE = {}


def build_nc(T, repeat=1, trace_sim=False):
    NJ = T // QT
    NKT = T // P
    HALF = NJ // 2
    assert NJ >= 2 and NJ % 2 == 0
    nc = bacc.Bacc("TRN2", target_bir_lowering=False, debug=False,
                   num_devices=NCORES)

    names = [
        ("xT", [D, T]), ("wq", [D, HW]), ("wk", [D, HW]), ("wv", [D, HW]),
        ("wo", [HW, D]), ("bq", [P, NH]), ("bk", [P, NH]), ("bv", [1, HW]),
        ("invg2q", [P, NH]), ("invg2k", [P, NH]),
        ("tab_cos", [P, T]), ("tab_sin", [P, T]), ("ones", [P, 1]),
    ]
    ap = {}
    for name, shape in names:
        ap[name] = nc.dram_tensor(name, shape, F32, kind="ExternalInput").ap()
    out_p = nc.dram_tensor("out_p", [T, D], F32, kind="ExternalOutput").ap()

    DBG = bool(os.environ.get("KERNEL_DEBUG"))
    dbg = {}
    if DBG:
        for nm, shape in (("dbg_yq", [P, NH, T]), ("dbg_yk", [P, NH, T]),
                          ("dbg_rq", [P, NH, T]), ("dbg_rk", [P, NH, T]),
                          ("dbg_v", [P, NKT * HW]), ("dbg_s", [2, T]),
                          ("dbg_den", [NH, T]), ("dbg_pv", [P, NH, T])):
            dbg[nm] = nc.dram_tensor(nm, shape, F32, kind="ExternalOutput").ap()

    xT_r = ap["xT"].rearrange("(o p) t -> p o t", p=P)       # [128, 16, T]
    wq_r = ap["wq"].rearrange("(o p) c -> p o c", p=P)       # [128, 16, 256]
    wk_r = ap["wk"].rearrange("(o p) c -> p o c", p=P)
    wv_r = ap["wv"].rearrange("(o p) c -> p o c", p=P)
    wo_r = ap["wo"].rearrange("(h p) d -> p h d", p=P)       # [128, 2, D]

    def _emit(tc, ctx):
        nc = tc.nc
        singles = ctx.enter_context(tc.tile_pool(name="singles", bufs=1))
        dram = ctx.enter_context(
            tc.tile_pool(name="dram", bufs=1, space="DRAM"))

        # tiny constants
        bq_sb = singles.tile([P, NH], F32)
        nc.sync.dma_start(bq_sb[:], ap["bq"][:])
        bk_sb = singles.tile([P, NH], F32)
        nc.sync.dma_start(bk_sb[:], ap["bk"][:])
        ivq_sb = singles.tile([P, NH], F32R)
        nc.sync.dma_start(ivq_sb[:], ap["invg2q"][:].bitcast(F32R))
        ivk_sb = singles.tile([P, NH], F32R)
        nc.sync.dma_start(ivk_sb[:], ap["invg2k"][:].bitcast(F32R))
        ones_sb = singles.tile([P, 1], F32R)
        nc.sync.dma_start(ones_sb[:], ap["ones"][:].bitcast(F32R))
        bv_bc = singles.tile([P, HW], F32)
        nc.gpsimd.dma_start(bv_bc[:], ap["bv"][:].to_broadcast([P, HW]))
        eps_sb = singles.tile([P, 1], F32)
        nc.vector.memset(eps_sb[:], EPS)

        # resident activations (per-j q/k tiles for precise dependencies)
        yq_j, yk_j = [], []
        ypool = ctx.enter_context(tc.tile_pool(name="ypool", bufs=1))
        for j in range(NJ):
            yq_j.append(ypool.tile([P, NH, QT], F32R,
                                   tag=f"yq{j}", name=f"yq{j}"))
            yk_j.append(ypool.tile([P, NH, QT], F32R,
                                   tag=f"yk{j}", name=f"yk{j}"))
        v_sb = ypool.tile([P, NKT, HW], F32R, tag="v")

        # per-half collective bounce buffers + rsqrt factors
        cc_in_h, cc_out_h, s_dram_h, s_pk_h = [], [], [], []
        for hf in range(2):
            cc_in_h.append(dram.tile([2, T // 2], F32, tag=f"cci{hf}",
                                     name=f"cci{hf}"))
            cc_out_h.append(dram.tile([2, T // 2], F32, tag=f"cco{hf}",
                                      name=f"cco{hf}"))
            s_dram_h.append(dram.tile([2, T // 2], F32, tag=f"sdr{hf}",
                                      name=f"sdr{hf}"))
            s_pk_h.append(singles.tile([P, 2, T // (2 * P)], F32,
                                       tag=f"spk{hf}", name=f"spk{hf}"))

        def emit_collective(hf):
            if os.environ.get("KERNEL_NO_CC"):
                nc.sync.dma_start(cc_out_h[hf][:], cc_in_h[hf][:])
            else:
                nc.gpsimd.collective_compute(
                    "AllReduce", mybir.AluOpType.add,
                    replica_groups=[list(range(NCORES))],
                    ins=[cc_in_h[hf].opt()], outs=[cc_out_h[hf].opt()])
            # s = rsqrt(ssq/D + eps); fold 1/sqrt(HD) into the q row
            s_pk = s_pk_h[hf]
            nc.sync.dma_start(
                s_pk[:], cc_out_h[hf][:].rearrange("r (c p) -> p r c", p=P))
            nc.scalar.activation(s_pk[:], s_pk[:], AF.Sqrt,
                                 bias=eps_sb[:, 0:1], scale=1.0 / D)
            nc.vector.reciprocal(s_pk[:], s_pk[:])
            nc.vector.tensor_scalar_mul(
                s_pk[:, 0, :], s_pk[:, 0, :], 1.0 / math.sqrt(HD))
            nc.sync.dma_start(
                s_dram_h[hf][:].rearrange("r (c p) -> p r c", p=P), s_pk[:])
            if DBG:
                nc.sync.dma_start(
                    dbg["dbg_s"][:, hf * (T // 2):(hf + 1) * (T // 2)],
                    s_dram_h[hf][:])

        # ---------------- Phase A: projections + ssq ----------------
        with tc.tile_pool(name="wpool", bufs=1) as wpool, \
             tc.tile_pool(name="xtpool", bufs=2) as xtpool, \
             tc.tile_pool(name="sqpool", bufs=2) as sqpool, \
             tc.tile_pool(name="ssqcp", bufs=2) as ssqcp, \
             tc.tile_pool(name="qkps", bufs=4, space="PSUM") as qkps, \
             tc.tile_pool(name="vps", bufs=4, space="PSUM") as vps:

            wq_sb = wpool.tile([P, NKC, HW], F32R)
            nc.sync.dma_start(wq_sb[:], wq_r.bitcast(F32R))
            wk_sb = wpool.tile([P, NKC, HW], F32R)
            nc.sync.dma_start(wk_sb[:], wk_r.bitcast(F32R))
            wv_sb = wpool.tile([P, NKC, HW], F32R)
            nc.sync.dma_start(wv_sb[:], wv_r.bitcast(F32R))

            for j in range(NJ):
                jsl = bass.ts(j, QT)
                hf = j // HALF
                jloc = bass.ds(j * QT - hf * (T // 2), QT)

                qk_ps = {}
                for tn in range(2):          # 0 = q, 1 = k
                    for h in range(NH):
                        qk_ps[tn, h] = qkps.tile(
                            [P, QT], F32, tag="qk", name=f"qk{tn}{h}")
                v_ps = [vps.tile([P, HW], F32, tag="v", name=f"v{tp}")
                        for tp in range(4)]

                # stream xT in 4 pieces; consume each piece fully so the
                # 2-slot xt pool never deadlocks the in-order PE
                for g in range(4):
                    xg = xtpool.tile([P, 4, QT], F32R, tag="xt")
                    nc.sync.dma_start(
                        xg[:], xT_r[:, 4 * g:4 * g + 4, jsl].bitcast(F32R))
                    for ol in range(4):
                        o = 4 * g + ol
                        st, sp = (o == 0), (o == NKC - 1)
                        for tn, w_sb in ((0, wq_sb), (1, wk_sb)):
                            for h in range(NH):
                                nc.tensor.matmul(
                                    qk_ps[tn, h][:],
                                    w_sb[:, o, h * HD:(h + 1) * HD],
                                    xg[:, ol, :], start=st, stop=sp)
                        for tp in range(4):
                            nc.tensor.matmul(
                                v_ps[tp][:], xg[:, ol, bass.ts(tp, P)],
                                wv_sb[:, o, :], start=st, stop=sp)

                # epilogues: bias add, squares, weighted ssq partition-sum
                for (tn, y_j, b_sb, iv_sb) in (
                        (0, yq_j, bq_sb, ivq_sb), (1, yk_j, bk_sb, ivk_sb)):
                    ssq_ps = qkps.tile([P, QT], F32, tag="qk",
                                       name=f"ssq{tn}")
                    for h in range(NH):
                        ytile = y_j[j][:, h, :]
                        nc.vector.tensor_scalar_add(
                            ytile, qk_ps[tn, h][:], b_sb[:, h:h + 1])
                        sqt = sqpool.tile([P, QT], F32R, tag="sq")
                        nc.vector.tensor_mul(sqt[:], ytile, ytile)
                        nc.tensor.matmul(
                            ssq_ps[0:1, :], iv_sb[:, h:h + 1], sqt[:],
                            start=(h == 0), stop=(h == NH - 1))
                    sscp = ssqcp.tile([1, QT], F32, tag=f"sscp{tn}")
                    nc.vector.tensor_copy(sscp[:], ssq_ps[0:1, :])
                    nc.sync.dma_start(cc_in_h[hf][tn:tn + 1, jloc], sscp[:])

                for tp in range(4):
                    nc.vector.tensor_add(
                        v_sb[:, 4 * j + tp, :], v_ps[tp][:], bv_bc[:])
                if DBG:
                    nc.sync.dma_start(
                        dbg["dbg_yq"][:, :, jsl], yq_j[j][:].bitcast(F32))
                    nc.sync.dma_start(
                        dbg["dbg_yk"][:, :, jsl], yk_j[j][:].bitcast(F32))

                if j == HALF - 1:
                    emit_collective(0)

        emit_collective(1)

        post = ctx.enter_context(tc.tile_pool(name="post", bufs=1))
        wo_sb = post.tile([P, NH, D], F32R)
        nc.sync.dma_start(wo_sb[:], wo_r.bitcast(F32R))
        if DBG:
            nc.sync.dma_start(dbg["dbg_v"][:], v_sb[:].bitcast(F32))

        # ---------------- Phase B/C/D: rope, attention, out-proj ----
        with tc.tile_pool(name="tabp", bufs=2) as tabp, \
             tc.tile_pool(name="swp", bufs=2) as swp, \
             tc.tile_pool(name="tmpp", bufs=2) as tmpp, \
             tc.tile_pool(name="bcp", bufs=2) as bcp, \
             tc.tile_pool(name="exp", bufs=2) as exp_pool, \
             tc.tile_pool(name="odp", bufs=5) as odp, \
             tc.tile_pool(name="outp", bufs=3) as outp, \
             tc.tile_pool(name="denp", bufs=2) as denp, \
             tc.tile_pool(name="scps", bufs=2, space="PSUM") as scps, \
             tc.tile_pool(name="pvps", bufs=2, space="PSUM") as pvps, \
             tc.tile_pool(name="dps", bufs=2, space="PSUM") as dps:

            def emit_rope(j):
                jsl = bass.ts(j, QT)
                hf = j // HALF
                jloc = slice(j * QT - hf * (T // 2),
                             (j + 1) * QT - hf * (T // 2))
                tc_t = tabp.tile([P, QT], F32, tag="tc", name="tc_t")
                nc.sync.dma_start(tc_t[:], ap["tab_cos"][:, jsl])
                ts_t = tabp.tile([P, QT], F32, tag="ts", name="ts_t")
                nc.sync.dma_start(ts_t[:], ap["tab_sin"][:, jsl])
                bc_q = bcp.tile([P, QT], F32, tag="bcq", name="bc_q")
                nc.gpsimd.dma_start(
                    bc_q[:], s_dram_h[hf][0:1, jloc].to_broadcast([P, QT]))
                bc_k = bcp.tile([P, QT], F32, tag="bck", name="bc_k")
                nc.gpsimd.dma_start(
                    bc_k[:], s_dram_h[hf][1:2, jloc].to_broadcast([P, QT]))
                for (y_j, bc) in ((yq_j, bc_q), (yk_j, bc_k)):
                    for h in range(NH):
                        ytile = y_j[j][:, h, :]
                        yf32 = ytile.bitcast(F32)
                        sw = swp.tile([P, QT], F32, tag="sw", name="sw")
                        nc.sync.dma_start(sw[0:P:2, :], yf32[1:P:2, :])
                        nc.sync.dma_start(sw[1:P:2, :], yf32[0:P:2, :])
                        tmp = tmpp.tile([P, QT], F32, tag="tmp", name="tmp")
                        nc.vector.tensor_mul(tmp[:], sw[:], ts_t[:])
                        nc.vector.tensor_mul(ytile, ytile, tc_t[:])
                        nc.vector.tensor_add(ytile, ytile, tmp[:])
                        nc.vector.tensor_mul(ytile, ytile, bc[:])
                if DBG:
                    nc.sync.dma_start(
                        dbg["dbg_rq"][:, :, jsl], yq_j[j][:].bitcast(F32))
                    nc.sync.dma_start(
                        dbg["dbg_rk"][:, :, jsl], yk_j[j][:].bitcast(F32))

            def emit_attention(j):
                od_h = []
                n_i = 4 * (j + 1)
                for h in range(NH):
                    pv = pvps.tile([P, QT], F32, tag="pv", name="pv")
                    den = dps.tile([1, QT], F32, tag="den", name="den")
                    for grp in range(n_i // 2):
                        sc = scps.tile([P, 2, QT], F32, tag="mm", name="sc")
                        for s in range(2):
                            i = 2 * grp + s
                            nc.tensor.matmul(
                                sc[:, s, :],
                                yk_j[i // 4][:, h,
                                             (i % 4) * P:(i % 4 + 1) * P],
                                yq_j[j][:, h, :], start=True, stop=True)
                        ex = exp_pool.tile([P, 2, QT], F32R, tag="ex",
                                           name="ex")
                        nc.scalar.activation(ex[:], sc[:], AF.Exp,
                                             bias=0.0, scale=1.0)
                        if grp >= n_i // 2 - 2:
                            # diagonal: keep iff q' - 128*m - p >= 0
                            base = -P * (2 * grp - 4 * j)
                            nc.gpsimd.affine_select(
                                out=ex[:], in_=ex[:],
                                compare_op=mybir.AluOpType.is_ge,
                                fill=0.0, base=base,
                                pattern=[[-P, 2], [1, QT]],
                                channel_multiplier=-1)
                        for s in range(2):
                            i = 2 * grp + s
                            nc.tensor.matmul(
                                pv[:], v_sb[:, i, h * HD:(h + 1) * HD],
                                ex[:, s, :],
                                start=(i == 0), stop=(i == n_i - 1))
                            nc.tensor.matmul(
                                den[0:1, :], ones_sb[:], ex[:, s, :],
                                start=(i == 0), stop=(i == n_i - 1))
                    if DBG:
                        jsl = bass.ts(j, QT)
                        dcp = denp.tile([1, QT], F32, tag="dcp", name="dcp")
                        nc.vector.tensor_copy(dcp[:], den[0:1, :])
                        nc.sync.dma_start(dbg["dbg_den"][h:h + 1, jsl],
                                          dcp[:])
                        pcp = outp.tile([P, QT], F32, tag="pcp", name="pcp")
                        nc.vector.tensor_copy(pcp[:], pv[:])
                        nc.sync.dma_start(dbg["dbg_pv"][:, h, jsl], pcp[:])
                    rden = denp.tile([1, QT], F32, tag="rden", name="rden")
                    nc.vector.reciprocal(rden[:], den[0:1, :])
                    rbc = bcp.tile([P, QT], F32, tag="rbc", name="rbc")
                    nc.gpsimd.partition_broadcast(rbc[:], rden[0:1, :])
                    od = odp.tile([P, QT], F32R, tag="od", name="od")
                    nc.vector.tensor_mul(od[:], pv[:], rbc[:])
                    od_h.append(od)
                return od_h

            def emit_outproj(j, od_h):
                for tp in range(4):
                    tsl = bass.ts(tp, P)
                    for dd in range(0, 4, 2):
                        ops = scps.tile([P, 2, QT], F32, tag="mm",
                                        name="ops")
                        for s2 in range(2):
                            dsl = bass.ts(dd + s2, QT)
                            for h in range(NH):
                                nc.tensor.matmul(
                                    ops[:, s2, :], od_h[h][:, tsl],
                                    wo_sb[:, h, dsl],
                                    start=(h == 0), stop=(h == NH - 1))
                        ot = outp.tile([P, 2, QT], F32, tag="ot", name="ot")
                        if dd == 0:
                            nc.scalar.activation(ot[:], ops[:], AF.Copy)
                        else:
                            nc.vector.tensor_copy(ot[:], ops[:])
                        nc.sync.dma_start(
                            out_p[j * QT + tp * P:j * QT + (tp + 1) * P,
                                  dd * QT:(dd + 2) * QT], ot[:])

            od_prev = None
            for j in range(NJ):
                emit_rope(j)
                od_now = emit_attention(j)
                if od_prev is not None:
                    emit_outproj(j - 1, od_prev)
                od_prev = od_now
            emit_outproj(NJ - 1, od_prev)

    with tile.TileContext(nc, trace_sim=trace_sim) as tc:
        for _rep in range(repeat):
            with ExitStack() as ctx:
                _emit(tc, ctx)

    nc.compile()
    return nc


def _prep_inputs(inputs, T):
    x = np.asarray(inputs["x"], np.float32)[0, :T]          # [T, D]
    freqs = np.asarray(inputs["freqs"], np.float32)[:T]     # [T, HD//2]
    xT = np.ascontiguousarray(x.T)                          # [D, T]

    cos = np.cos(freqs)                                     # [T, 64]
    sin = np.sin(freqs)
    tab_cos = np.ascontiguousarray(np.repeat(cos.T, 2, axis=0))  # [128, T]
    tab_sin = np.empty((HD, T), np.float32)
    tab_sin[0::2] = -sin.T
    tab_sin[1::2] = sin.T

    ones = np.ones((P, 1), np.float32)

    in_maps = []
    for c in range(NCORES):
        hsl = slice(c * HW, (c + 1) * HW)
        gq = np.asarray(inputs["gq"], np.float32)[hsl]
        gk = np.asarray(inputs["gk"], np.float32)[hsl]
        wq = np.asarray(inputs["Wq"], np.float32)[:, hsl] * gq[None, :]
        wk = np.asarray(inputs["Wk"], np.float32)[:, hsl] * gk[None, :]
        wv = np.ascontiguousarray(np.asarray(inputs["Wv"], np.float32)[:, hsl])
        wo = np.ascontiguousarray(np.asarray(inputs["Wo"], np.float32)[hsl, :])
        bq = np.asarray(inputs["bq"], np.float32)[hsl] * gq
        bk = np.asarray(inputs["bk"], np.float32)[hsl] * gk
        bv = np.asarray(inputs["bv"], np.float32)[hsl]
        in_maps.append({
            "xT": xT,
            "wq": np.ascontiguousarray(wq),
            "wk": np.ascontiguousarray(wk),
            "wv": wv, "wo": wo,
            "bq": np.ascontiguousarray(bq.reshape(NH, P).T),
            "bk": np.ascontiguousarray(bk.reshape(NH, P).T),
            "bv": bv.reshape(1, HW),
            "invg2q": np.ascontiguousarray(
                (1.0 / np.square(gq)).reshape(NH, P).T.astype(np.float32)),
            "invg2k": np.ascontiguousarray(
                (1.0 / np.square(gk)).reshape(NH, P).T.astype(np.float32)),
            "tab_cos": tab_cos, "tab_sin": tab_sin, "ones": ones,
        })
    return in_maps


def _run(inputs, T=T_FULL, trace=False, **spmd_kwargs):
    if T not in _NC_CACHE:
        _NC_CACHE[T] = build_nc(T)
    nc = _NC_CACHE[T]
    in_maps = _prep_inputs(inputs, T)
    res = run_bass_kernel_spmd(nc, in_maps, list(range(NCORES)),
                               trace=trace, **spmd_kwargs)
    acc = np.zeros((T, D), np.float64)
    for c in range(NCORES):
        acc += res.results[c]["out_p"]
    acc += np.asarray(inputs["bo"], np.float64)[None, :]
    out = acc.astype(np.float32)[None]
    return out, res


def kernel(**inputs) -> np.ndarray:
    out, _ = _run(inputs)
    return out

